# revision 1
# baseline (speedup 1.0000x reference)
"""TRN2 Bass kernel: cross-attention (nn_CrossAttention_42047729828228).

Computes, per batch b:
  q = x @ Wq.T ; k = key @ Wk.T ; v = value @ Wv.T      (heads H=8, C=64)
  sim = einsum('nhc,mhc->hnm', q, k) * SCALE
  sim = where(mask, sim, -inf) + L1*box + L2*road
  out = einsum('hnm,mhc->nhc', softmax(sim, -1), v) @ Wo.T + bo

Device strategy: data-parallel over batch B=32 across 8 NeuronCores (4 each).
Per core, one Tile program processes its 4 batches.

Key algebraic simplifications (exact):
 - road bias is constant along the softmax (key) axis -> cancels; dropped.
 - SCALE folded into Wq host-side.
 - key mask folded into an additive per-key bias (0 / -1e9), applied with the
   box bias in one fused DVE op; exp(-1e9) == 0 exactly in fp32.

On-chip layout: scores are built transposed, simT (m on partitions, n free),
so the sim matmul streams n (free dim 512 -> full-rate fp32r) and the AV
matmul consumes exp(simT) directly as its moving operand. The softmax
denominator is accumulated into one (8, n) psum via one-hot lhsT matmuls,
reciprocal'd on DVE, broadcast back to 128 partitions with a constant
pair-selector matmul, and applied as the PSUM->SBUF move of the AV output.
"""

import os
import sys

import numpy as np

sys.path.insert(0, "/opt/trn_rl_repo")

import concourse.bass as bass  # noqa: E402
import concourse.bacc as bacc  # noqa: E402
import concourse.mybir as mybir  # noqa: E402
import concourse.tile as tile  # noqa: E402

F32 = mybir.dt.float32
F32R = mybir.dt.float32r
AF = mybir.ActivationFunctionType
ALU = mybir.AluOpType

# Problem shapes (hardcoded; see module docstring).
B, N, M = 32, 1536, 80
QD, KD, VD = 320, 768, 768
H, C = 8, 64
INNER = H * C  # 512
OD = QD  # 320
SCALE = C**-0.5
NCORES = 8
BP = B // NCORES  # 4 batches per core
NCH = 512  # n-chunk (matmul moving dim)
NT = 128  # n-tile
NCHUNKS = N // NCH  # 3
NTT = NCH // NT  # 4
NPAIR = H // 2  # 4 head pairs
IC = INNER // 128  # 4 i-chunks
KC = KD // 128  # 6 kd-chunks
MASK_NEG = -1.0e9


def build_program(split_waits=True):  # split_waits kept for API compat; Bacc.compile() handles it
    nc = bacc.Bacc("TRN2", target_bir_lowering=False, debug=False, num_devices=NCORES)

    x_d = nc.dram_tensor("x", [BP, N, QD], F32, kind="ExternalInput").ap()
    key_d = nc.dram_tensor("key", [BP, M, KD], F32, kind="ExternalInput").ap()
    val_d = nc.dram_tensor("value", [BP, M, VD], F32, kind="ExternalInput").ap()
    mb_d = nc.dram_tensor("maskbias", [BP, M], F32, kind="ExternalInput").ap()
    box_d = nc.dram_tensor("box", [BP, N, M], F32, kind="ExternalInput").ap()
    wqt_d = nc.dram_tensor("WqT", [QD, INNER], F32R, kind="ExternalInput").ap()
    wkt_d = nc.dram_tensor("WkT", [KD, INNER], F32R, kind="ExternalInput").ap()
    wvt_d = nc.dram_tensor("WvT", [VD, INNER], F32R, kind="ExternalInput").ap()
    wot_d = nc.dram_tensor("WoT", [INNER, OD], F32R, kind="ExternalInput").ap()
    bo_d = nc.dram_tensor("bo", [OD], F32, kind="ExternalInput").ap()
    ident_d = nc.dram_tensor("ident", [128, 128], F32, kind="ExternalInput").ap()
    zoneh_d = nc.dram_tensor("zoneh", [M, H, H], F32R, kind="ExternalInput").ap()
    psel_d = nc.dram_tensor("psel", [H, NPAIR, 128], F32R, kind="ExternalInput").ap()
    out_d = nc.dram_tensor("out", [BP, N, OD], F32, kind="ExternalOutput").ap()

    with tile.TileContext(nc) as tc:
        with (
            tc.tile_pool(name="wpool", bufs=1) as wp,
            tc.tile_pool(name="bpool", bufs=2) as bp,
            tc.tile_pool(name="cpool", bufs=2) as cp,
            tc.tile_pool(name="tp", bufs=2, space="PSUM") as tp,
            tc.tile_pool(name="pq", bufs=1, space="PSUM") as pq_pool,
            tc.tile_pool(name="ps", bufs=2, space="PSUM") as ps,
            tc.tile_pool(name="pd", bufs=1, space="PSUM") as pd_pool,
            tc.tile_pool(name="po", bufs=2, space="PSUM") as po,
        ):
            # --- one-time loads ---
            wq_a = wp.tile([128, 2, INNER], F32R)
            nc.sync.dma_start(wq_a, wqt_d[0:256].rearrange("(c p) i -> p c i", p=128))
            wq_b = wp.tile([64, INNER], F32R)
            nc.sync.dma_start(wq_b, wqt_d[256:QD])
            wk_sb = wp.tile([128, KC, INNER], F32R)
            nc.sync.dma_start(wk_sb, wkt_d.rearrange("(c p) i -> p c i", p=128))
            wv_sb = wp.tile([128, KC, INNER], F32R)
            nc.sync.dma_start(wv_sb, wvt_d.rearrange("(c p) i -> p c i", p=128))
            wo_sb = wp.tile([128, IC, OD], F32R)
            nc.sync.dma_start(wo_sb, wot_d.rearrange("(c p) o -> p c o", p=128))
            ident = wp.tile([128, 128], F32)
            nc.sync.dma_start(ident, ident_d)
            zoneh = wp.tile([M, H, H], F32R)
            nc.sync.dma_start(zoneh, zoneh_d)
            psel = wp.tile([H, NPAIR, 128], F32R)
            nc.sync.dma_start(psel, psel_d)
            bo_sb = wp.tile([128, OD], F32)
            nc.sync.dma_start(bo_sb, bo_d[None, :].to_broadcast([128, OD]))

            for b in range(BP):
                # --- per-batch K/V stage ---
                key_sb = bp.tile([M, KD], F32, tag="key_sb")
                nc.sync.dma_start(key_sb, key_d[b])
                val_sb = bp.tile([M, VD], F32, tag="val_sb")
                nc.sync.dma_start(val_sb, val_d[b])
                mb_sb = bp.tile([M, 1], F32, tag="mb_sb")
                nc.sync.dma_start(mb_sb, mb_d[b][:, None])

                keyT = bp.tile([128, KC, M], F32R, tag="keyT")
                valT = bp.tile([128, KC, M], F32R, tag="valT")
                for kc in range(KC):
                    pt = tp.tile([128, M], F32, tag="tp")
                    nc.tensor.transpose(
                        pt, key_sb[:, kc * 128 : (kc + 1) * 128], ident[:M, :M]
                    )
                    nc.any.tensor_copy(keyT[:, kc, :], pt)
                    pt2 = tp.tile([128, M], F32, tag="tp")
                    nc.tensor.transpose(
                        pt2, val_sb[:, kc * 128 : (kc + 1) * 128], ident[:M, :M]
                    )
                    nc.any.tensor_copy(valT[:, kc, :], pt2)

                pk = ps.tile([M, INNER], F32, tag="ps")
                for kc in range(KC):
                    nc.tensor.matmul(
                        pk,
                        keyT[:, kc, :],
                        wk_sb[:, kc, :],
                        start=(kc == 0),
                        stop=(kc == KC - 1),
                    )
                k_sb = bp.tile([M, INNER], F32, tag="k_sb")
                nc.any.tensor_copy(k_sb, pk)
                kT = bp.tile([128, IC, M], F32R, tag="kT")
                for ic in range(IC):
                    pt = tp.tile([128, M], F32, tag="tp")
                    nc.tensor.transpose(
                        pt, k_sb[:, ic * 128 : (ic + 1) * 128], ident[:M, :M]
                    )
                    nc.any.tensor_copy(kT[:, ic, :], pt)

                pv = ps.tile([M, INNER], F32, tag="ps")
                for kc in range(KC):
                    nc.tensor.matmul(
                        pv,
                        valT[:, kc, :],
                        wv_sb[:, kc, :],
                        start=(kc == 0),
                        stop=(kc == KC - 1),
                    )
                v_sb = bp.tile([M, INNER], F32R, tag="v_sb")
                nc.any.tensor_copy(v_sb, pv)

                # --- per-chunk pipeline ---
                for j in range(NCHUNKS):
                    nsl = slice(j * NCH, (j + 1) * NCH)
                    x_sb = cp.tile([128, NTT, QD], F32, tag="x_sb")
                    nc.sync.dma_start(
                        x_sb, x_d[b, nsl, :].rearrange("(t p) q -> p t q", p=128)
                    )
                    box_sb = cp.tile([128, NTT, M], F32, tag="box_sb")
                    nc.sync.dma_start(
                        box_sb, box_d[b, nsl, :].rearrange("(t p) m -> p t m", p=128)
                    )

                    xT0 = cp.tile([128, NCH], F32R, tag="xT0")
                    xT1 = cp.tile([128, NCH], F32R, tag="xT1")
                    xT2 = cp.tile([64, NCH], F32R, tag="xT2")
                    for t in range(NTT):
                        for lo, w, dst in ((0, 128, xT0), (128, 128, xT1), (256, 64, xT2)):
                            pt = tp.tile([w, 128], F32, tag="tp")
                            nc.tensor.transpose(pt, x_sb[:, t, lo : lo + w], ident)
                            nc.any.tensor_copy(dst[:, t * 128 : (t + 1) * 128], pt)

                    qT = cp.tile([128, IC, NCH], F32R, tag="qT")
                    for ic in range(IC):
                        pq = pq_pool.tile([128, NCH], F32, tag="pq")
                        isl = slice(ic * 128, (ic + 1) * 128)
                        nc.tensor.matmul(
                            pq, wq_a[:, 0, isl], xT0, start=True, stop=False
                        )
                        nc.tensor.matmul(
                            pq, wq_a[:, 1, isl], xT1, start=False, stop=False
                        )
                        nc.tensor.matmul(
                            pq, wq_b[:, isl], xT2, start=False, stop=True
                        )
                        nc.any.tensor_copy(qT[:, ic, :], pq)

                    boxT5 = cp.tile([M, NCH], F32, tag="boxT5")
                    for t in range(NTT):
                        pt = tp.tile([M, 128], F32, tag="tp")
                        nc.tensor.transpose(pt, box_sb[:, t, :], ident)
                        nc.any.tensor_copy(boxT5[:, t * 128 : (t + 1) * 128], pt)

                    e_all = cp.tile([M, H, NCH], F32R, tag="e_all")
                    pd = pd_pool.tile([H, NCH], F32, tag="pd")
                    for h in range(H):
                        pss = ps.tile([M, NCH], F32, tag="ps")
                        r0 = (h % 2) * 64
                        nc.tensor.matmul(
                            pss,
                            kT[r0 : r0 + 64, h // 2, :],
                            qT[r0 : r0 + 64, h // 2, :],
                            start=True,
                            stop=True,
                        )
                        nc.vector.scalar_tensor_tensor(
                            out=pss,
                            in0=pss,
                            scalar=mb_sb,
                            in1=boxT5,
                            op0=ALU.add,
                            op1=ALU.add,
                        )
                        nc.scalar.activation(e_all[:, h, :], pss, AF.Exp)
                        nc.tensor.matmul(
                            pd,
                            zoneh[:, h, :],
                            e_all[:, h, :],
                            start=(h == 0),
                            stop=(h == H - 1),
                        )

                    recip = cp.tile([H, NCH], F32R, tag="recip")
                    with nc.allow_low_precision(reason="fp32r softmax denom"):
                        nc.vector.reciprocal(recip, pd)

                    o_all = cp.tile([128, NPAIR, NCH], F32R, tag="o_all")
                    for p in range(NPAIR):
                        prb = po.tile([128, NCH], F32, tag="po")
                        nc.tensor.matmul(
                            prb, psel[:, p, :], recip, start=True, stop=True
                        )
                        rb_sb = cp.tile([128, NCH], F32, tag="rb_sb")
                        nc.scalar.copy(rb_sb, prb)
                        for r, h in ((0, 2 * p), (64, 2 * p + 1)):
                            pav = po.tile([128, NCH], F32, tag="po")
                            nc.tensor.matmul(
                                pav[0:64, :],
                                v_sb[:, h * C : (h + 1) * C],
                                e_all[:, h, :],
                                start=True,
                                stop=True,
                            )
                            nc.vector.tensor_tensor(
                                o_all[r : r + 64, p, :],
                                pav[0:64, :],
                                rb_sb[r : r + 64, :],
                                ALU.mult,
                            )

                    out_sb = cp.tile([128, NTT, OD], F32, tag="out_sb")
                    for t in range(NTT):
                        pf = pq_pool.tile([128, OD], F32, tag="pq")
                        for ic in range(IC):
                            nc.tensor.matmul(
                                pf,
                                o_all[:, ic, t * 128 : (t + 1) * 128],
                                wo_sb[:, ic, :],
                                start=(ic == 0),
                                stop=(ic == IC - 1),
                            )
                        nc.vector.tensor_add(out_sb[:, t, :], pf, bo_sb)
                    nc.sync.dma_start(
                        out_d[b, nsl, :].rearrange("(t p) o -> p t o", p=128), out_sb
                    )
    nc.compile()
    return nc


def host_inputs(x, key, value, mask, perl_box_masking_map, perl_road_masking_map,
                Wq, Wk, Wv, Wo, bo):
    """Host-side input marshaling: weight transposes, constant tables, mask
    bias. The road bias cancels inside the softmax and is dropped."""
    del perl_road_masking_map
    x = np.ascontiguousarray(np.asarray(x, np.float32))
    key = np.ascontiguousarray(np.asarray(key, np.float32))
    value = np.ascontiguousarray(np.asarray(value, np.float32))
    box = np.ascontiguousarray(np.asarray(perl_box_masking_map, np.float32) * np.float32(5.0))
    mask = np.asarray(mask, bool)
    maskbias = np.where(mask, np.float32(0.0), np.float32(MASK_NEG))
    maskbias = np.ascontiguousarray(maskbias.astype(np.float32))

    wqt = np.ascontiguousarray((np.asarray(Wq, np.float32) * np.float32(SCALE)).T)
    wkt = np.ascontiguousarray(np.asarray(Wk, np.float32).T)
    wvt = np.ascontiguousarray(np.asarray(Wv, np.float32).T)
    wot = np.ascontiguousarray(np.asarray(Wo, np.float32).T)
    bo = np.ascontiguousarray(np.asarray(bo, np.float32))

    ident = np.eye(128, dtype=np.float32)
    zoneh = np.zeros((M, H, H), np.float32)
    for h in range(H):
        zoneh[:, h, h] = 1.0
    psel = np.zeros((H, NPAIR, 128), np.float32)
    for p in range(NPAIR):
        psel[2 * p, p, 0:64] = 1.0
        psel[2 * p + 1, p, 64:128] = 1.0

    shared = {
        "WqT": wqt, "WkT": wkt, "WvT": wvt, "WoT": wot, "bo": bo,
        "ident": ident, "zoneh": zoneh, "psel": psel,
    }
    in_maps = []
    for c in range(NCORES):
        sl = slice(c * BP, (c + 1) * BP)
        m = {
            "x": x[sl], "key": key[sl], "value": value[sl],
            "maskbias": maskbias[sl], "box": box[sl],
        }
        m.update(shared)
        in_maps.append(m)
    return in_maps


_PROGRAM = None
LAST_RESULT = None


def kernel(**inputs):
    global _PROGRAM, LAST_RESULT
    from concourse.bass_utils import run_bass_kernel_spmd

    if _PROGRAM is None:
        _PROGRAM = build_program()
    in_maps = host_inputs(**inputs)
    trace = bool(int(os.environ.get("KERNEL_TRACE", "0")))
    res = run_bass_kernel_spmd(
        _PROGRAM, in_maps, list(range(NCORES)), trace=trace
    )
    LAST_RESULT = res
    out = np.concatenate([res.results[c]["out"] for c in range(NCORES)], axis=0)
    return np.ascontiguousarray(out.astype(np.float32))



# revision 29
# speedup vs baseline: 1.4143x; 1.4143x over previous
"""TRN2 Bass kernel: cross-attention (nn_CrossAttention_42047729828228).

Computes, per batch b:
  q = x @ Wq.T ; k = key @ Wk.T ; v = value @ Wv.T      (heads H=8, C=64)
  sim = einsum('nhc,mhc->hnm', q, k) * SCALE
  sim = where(mask, sim, -inf) + L1*box + L2*road
  out = einsum('hnm,mhc->nhc', softmax(sim, -1), v) @ Wo.T + bo

Device strategy: data-parallel over batch B=32 across 8 NeuronCores (4 each).

Host-side algebraic prep (exact or within bf16/fp8 tolerance):
 - road bias is constant along the softmax (key) axis -> cancels; dropped.
 - SCALE folded into Wq.
 - mask and box bias folded into one multiplicative table
   eboxT[m, n] = exp(L1*box[n, m] + maskbias[m]) (bf16), so on device
   e = eboxT * exp(qk) needs one fast all-SBUF bf16 multiply per head.
 - x, key, value shipped pre-transposed (feature-major): zero PE transposes.
 - x/Wq and key/Wk ship as fp8e4 in DoubleRow-packed pairs, scaled by 64 to
   clear the fp8 subnormal floor; the resulting 4096x on sim is removed
   exactly by exp(scale=2^-12) on the Activation engine. q/k quantization
   error is softmax-damped, so fp8 is safe here (and only here).

On-chip layout: scores are built transposed, simT (m on partitions, n free);
the AV matmul consumes e directly as its moving operand and two heads share
each AV psum bank. Unnormalized AV pairs evacuate immediately via Pool; the
softmax reciprocal is bounced through a DRAM scratch and partition-broadcast
back by DMA, making the normalize step a fast all-SBUF bf16 DVE multiply.
"""

import os
import sys

import numpy as np

sys.path.insert(0, "/opt/trn_rl_repo")

import concourse.bass as bass  # noqa: E402
import concourse.bacc as bacc  # noqa: E402
import concourse.mybir as mybir  # noqa: E402
import concourse.tile as tile  # noqa: E402

F32 = mybir.dt.float32
BF16 = mybir.dt.bfloat16
FP8 = mybir.dt.float8e4
AF = mybir.ActivationFunctionType
ALU = mybir.AluOpType
DR = mybir.MatmulPerfMode.DoubleRow

# Problem shapes (hardcoded; see module docstring).
B, N, M = 32, 1536, 80
QD, KD, VD = 320, 768, 768
H, C = 8, 64
INNER = H * C  # 512
OD = QD  # 320
SCALE = C**-0.5
NCORES = 8
BP = B // NCORES  # 4 batches per core
NCH = 512  # n-chunk (matmul moving dim)
NCHUNKS = N // NCH  # 3
NTT = NCH // 128  # 4 n-tiles per chunk
NPAIR = H // 2  # 4 head pairs
IC = INNER // 128  # 4 inner-chunks
KC = KD // 128  # 6 kd-chunks (bf16 V path)
KP = KD // 256  # 3 packed kd-chunks (fp8 K path)
QP2 = QD // 2  # 160 packed qd pairs
MASK_NEG = -1.0e9
W8 = 64.0  # fp8 weight upscale; sim carries 64*64, removed in exp scale


def build_program(split_waits=True):  # split_waits kept for API compat
    nc = bacc.Bacc("TRN2", target_bir_lowering=False, debug=False, num_devices=NCORES)

    xp_d = nc.dram_tensor("xP", [BP, 80, 2, 2, N], FP8, kind="ExternalInput").ap()
    keyp_d = nc.dram_tensor("keyP", [BP, KD // 2, 2, M], FP8, kind="ExternalInput").ap()
    valt_d = nc.dram_tensor("valT", [BP, VD, M], BF16, kind="ExternalInput").ap()
    ebox_d = nc.dram_tensor("eboxT", [BP, M, N], BF16, kind="ExternalInput").ap()
    wqp_d = nc.dram_tensor("WqP", [80, 2, 2, INNER], FP8, kind="ExternalInput").ap()
    wkp_d = nc.dram_tensor("WkP", [KD // 2, 2, INNER], FP8, kind="ExternalInput").ap()
    wv_d = nc.dram_tensor("WvT", [VD, INNER], BF16, kind="ExternalInput").ap()
    wo_d = nc.dram_tensor("WoT", [INNER, OD], BF16, kind="ExternalInput").ap()
    bo_d = nc.dram_tensor("bo", [OD], F32, kind="ExternalInput").ap()
    zoneh_d = nc.dram_tensor("zoneh", [M, H, H], BF16, kind="ExternalInput").ap()
    psel_d = nc.dram_tensor("psel", [H, NPAIR, 128], BF16, kind="ExternalInput").ap()
    out_d = nc.dram_tensor("out", [BP, N, OD], F32, kind="ExternalOutput").ap()

    with tile.TileContext(nc) as tc:
        with (
            tc.tile_pool(name="wpool", bufs=1) as wp,
            tc.tile_pool(name="bpool", bufs=2) as bp,
            tc.tile_pool(name="cpool", bufs=2) as cp,
            tc.tile_pool(name="pq", bufs=2, space="PSUM") as pq_pool,
            tc.tile_pool(name="ps", bufs=2, space="PSUM") as ps,
            tc.tile_pool(name="po", bufs=2, space="PSUM") as po,
            tc.tile_pool(name="pf", bufs=2, space="PSUM") as pf_pool,
        ):
            # --- one-time loads (K/V-path weights first: they gate batch 0) ---
            wk_sb = wp.tile([128, KP, 2, INNER], FP8)
            nc.sync.dma_start(wk_sb, wkp_d.rearrange("(c p) i j -> p c i j", p=128))
            wv_sb = wp.tile([128, KC, INNER], BF16)
            nc.sync.dma_start(wv_sb, wv_d.rearrange("(c p) i -> p c i", p=128))
            wq_sb = wp.tile([80, 2, 2, INNER], FP8)
            nc.sync.dma_start(wq_sb, wqp_d)
            zoneh = wp.tile([M, H, H], BF16)
            nc.sync.dma_start(zoneh, zoneh_d)
            psel = wp.tile([H, NPAIR, 128], BF16)
            nc.sync.dma_start(psel, psel_d)
            wo_sb = wp.tile([128, IC, OD], BF16)
            nc.sync.dma_start(wo_sb, wo_d.rearrange("(c p) o -> p c o", p=128))
            bo_sb = wp.tile([128, OD], F32)
            nc.sync.dma_start(bo_sb, bo_d[None, :].to_broadcast([128, OD]))

            for b in range(BP):
                # --- per-batch K/V stage ---
                keyt_sb = bp.tile([128, KP, 2, M], FP8, tag="keyt_sb")
                nc.sync.dma_start(
                    keyt_sb, keyp_d[b].rearrange("(c p) i m -> p c i m", p=128)
                )
                valt_sb = bp.tile([128, KC, M], BF16, tag="valt_sb")
                nc.sync.dma_start(valt_sb, valt_d[b].rearrange("(c p) m -> p c m", p=128))

                # kT[c, m] directly via fp8 DoubleRow: lhsT=WkP, moving=keyP
                kT = bp.tile([128, IC, M], BF16, tag="kT")
                for ic in range(IC):
                    pkt = pq_pool.tile([128, M], F32, tag="pq")
                    isl = slice(ic * 128, (ic + 1) * 128)
                    for kc in range(KP):
                        nc.tensor.matmul(
                            pkt,
                            wk_sb[:, kc, :, isl],
                            keyt_sb[:, kc, :, :],
                            start=(kc == 0),
                            stop=(kc == KP - 1),
                            perf_mode=DR,
                        )
                    nc.scalar.copy(kT[:, ic, :], pkt)

                # v[m, inner]: lhsT=valT chunk (free 80), moving=Wv (free 512)
                pv = ps.tile([M, INNER], F32, tag="ps")
                for kc in range(KC):
                    nc.tensor.matmul(
                        pv,
                        valt_sb[:, kc, :],
                        wv_sb[:, kc, :],
                        start=(kc == 0),
                        stop=(kc == KC - 1),
                    )
                v_sb = bp.tile([M, INNER], BF16, tag="v_sb")
                nc.scalar.copy(v_sb, pv)

                # --- per-chunk pipeline ---
                for j in range(NCHUNKS):
                    nsl = slice(j * NCH, (j + 1) * NCH)
                    xt = cp.tile([80, 2, 2, NCH], FP8, tag="xt")
                    nc.sync.dma_start(xt, xp_d[b, :, :, :, nsl])
                    ebox_sb = cp.tile([M, NCH], BF16, tag="ebox_sb")
                    nc.sync.dma_start(ebox_sb, ebox_d[b, :, nsl])

                    # qT[ic, n] = WqP . xP  (fp8 DoubleRow, 2 passes of 160)
                    qT = cp.tile([128, IC, NCH], BF16, tag="qT")
                    for ic in range(IC):
                        pq = pq_pool.tile([128, NCH], F32, tag="pq")
                        isl = slice(ic * 128, (ic + 1) * 128)
                        for ps_ in range(2):
                            nc.tensor.matmul(
                                pq, wq_sb[:, ps_, :, isl], xt[:, ps_, :, :],
                                start=(ps_ == 0), stop=(ps_ == 1),
                                perf_mode=DR,
                            )
                        nc.scalar.copy(qT[:, ic, :], pq)

                    # per-head: sim -> exp(2^-12 x) -> *ebox -> denom -> AV.
                    # Two heads share one AV psum bank; pairs evacuate
                    # unnormalized via Pool as soon as both heads land.
                    e_qk = cp.tile([M, H, NCH], BF16, tag="e_qk")
                    e_all = cp.tile([M, H, NCH], BF16, tag="e_all")
                    o_un = cp.tile([128, NPAIR, NCH], BF16, tag="o_un")
                    pdn = pq_pool.tile([H, NCH], F32, tag="pq")
                    pav = None
                    for h in range(H):
                        pss = ps.tile([M, NCH], F32, tag="ps")
                        r0 = (h % 2) * 64
                        nc.tensor.matmul(
                            pss,
                            kT[r0 : r0 + 64, h // 2, :],
                            qT[r0 : r0 + 64, h // 2, :],
                            start=True,
                            stop=True,
                        )
                        nc.scalar.activation(
                            e_qk[:, h, :], pss, AF.Exp, scale=1.0 / (W8 * W8)
                        )
                        # all-SBUF multiply: the one op GPSIMD can legally run
                        nc.gpsimd.tensor_tensor(
                            e_all[:, h, :], e_qk[:, h, :], ebox_sb, ALU.mult
                        )
                        nc.tensor.matmul(
                            pdn,
                            zoneh[:, h, :],
                            e_all[:, h, :],
                            start=(h == 0),
                            stop=(h == H - 1),
                        )
                        if h % 2 == 0:
                            pav = po.tile([128, NCH], F32, tag="po")
                        nc.tensor.matmul(
                            pav[r0 : r0 + 64, :],
                            v_sb[:, h * C : (h + 1) * C],
                            e_all[:, h, :],
                            start=True,
                            stop=True,
                        )
                        if h % 2 == 1:
                            nc.vector.tensor_copy(o_un[:, h // 2, :], pav)

                    recip = cp.tile([H, NCH], BF16, tag="recip")
                    with nc.allow_low_precision(reason="bf16 softmax denom"):
                        nc.vector.reciprocal(recip, pdn)

                    # broadcast recip rows to 64-partition pair blocks on PE;
                    # the normalize multiply consumes the psum directly.
                    o_all = cp.tile([128, NPAIR, NCH], BF16, tag="o_all")
                    for p in range(NPAIR):
                        prb = po.tile([128, NCH], F32, tag="po")
                        nc.tensor.matmul(prb, psel[:, p, :], recip, start=True, stop=True)
                        nc.vector.tensor_tensor(
                            o_all[:, p, :], o_un[:, p, :], prb, ALU.mult
                        )

                    # out = o_all.T @ Wo + bo
                    out_sb = cp.tile([128, NTT, OD], F32, tag="out_sb")
                    for t in range(NTT):
                        pf = pf_pool.tile([128, OD], F32, tag="pf")
                        for ic in range(IC):
                            nc.tensor.matmul(
                                pf,
                                o_all[:, ic, t * 128 : (t + 1) * 128],
                                wo_sb[:, ic, :],
                                start=(ic == 0),
                                stop=(ic == IC - 1),
                            )
                        nc.vector.tensor_tensor(out_sb[:, t, :], pf, bo_sb, ALU.add)
                    nc.sync.dma_start(
                        out_d[b, nsl, :].rearrange("(t p) o -> p t o", p=128), out_sb
                    )
    nc.compile()
    return nc


def host_inputs(x, key, value, mask, perl_box_masking_map, perl_road_masking_map,
                Wq, Wk, Wv, Wo, bo):
    """Host-side input marshaling: transposes, bf16/fp8 casts, DoubleRow
    packing, exp(bias) table. Road bias cancels inside softmax; dropped."""
    import ml_dtypes

    bf16 = ml_dtypes.bfloat16
    fp8 = ml_dtypes.float8_e4m3
    del perl_road_masking_map
    x = np.asarray(x, np.float32)
    key = np.asarray(key, np.float32)
    value = np.asarray(value, np.float32)
    box = np.asarray(perl_box_masking_map, np.float32)
    mask = np.asarray(mask, bool)
    maskbias = np.where(mask, np.float32(0.0), np.float32(MASK_NEG))

    # e-bias table: exp(5*box[n,m] + maskbias[m]) transposed to [m, n]
    ebias = np.float32(5.0) * box + maskbias[:, None, :]
    eboxT = np.ascontiguousarray(
        np.exp(ebias, dtype=np.float32).transpose(0, 2, 1)
    ).astype(bf16)

    # fp8 DoubleRow packs: feature dim as (pass, partition, slot); one x DMA
    xT = x.transpose(0, 2, 1)  # [B, QD, N]; qd = pass*160 + p*2 + i
    xP = np.ascontiguousarray(
        xT.reshape(B, 2, 80, 2, N).transpose(0, 2, 1, 3, 4)
    ).astype(fp8)
    keyT = key.transpose(0, 2, 1)  # [B, KD, M]
    keyP = np.ascontiguousarray(keyT.reshape(B, KD // 2, 2, M)).astype(fp8)
    valT = np.ascontiguousarray(value.transpose(0, 2, 1)).astype(bf16)

    w8 = np.float32(W8)
    wqp = (np.asarray(Wq, np.float32) * np.float32(SCALE) * w8).T  # [QD, INNER]
    wqp = np.ascontiguousarray(
        wqp.reshape(2, 80, 2, INNER).transpose(1, 0, 2, 3)
    ).astype(fp8)
    wkp = (np.asarray(Wk, np.float32) * w8).T  # [KD, INNER]
    wkp = np.ascontiguousarray(wkp.reshape(KD // 2, 2, INNER)).astype(fp8)
    wvt = np.ascontiguousarray(np.asarray(Wv, np.float32).T).astype(bf16)
    wot = np.ascontiguousarray(np.asarray(Wo, np.float32).T).astype(bf16)
    bo = np.ascontiguousarray(np.asarray(bo, np.float32))

    zoneh = np.zeros((M, H, H), bf16)
    for h in range(H):
        zoneh[:, h, h] = 1.0
    psel = np.zeros((H, NPAIR, 128), bf16)
    for p in range(NPAIR):
        psel[2 * p, p, 0:64] = 1.0
        psel[2 * p + 1, p, 64:128] = 1.0

    shared = {
        "WqP": wqp, "WkP": wkp, "WvT": wvt, "WoT": wot, "bo": bo,
        "zoneh": zoneh, "psel": psel,
    }
    in_maps = []
    for c in range(NCORES):
        sl = slice(c * BP, (c + 1) * BP)
        m = {
            "xP": xP[sl], "keyP": keyP[sl], "valT": valT[sl],
            "eboxT": eboxT[sl],
        }
        m.update(shared)
        in_maps.append(m)
    return in_maps


_PROGRAM = None
LAST_RESULT = None


def kernel(**inputs):
    global _PROGRAM, LAST_RESULT
    from concourse.bass_utils import run_bass_kernel_spmd

    if _PROGRAM is None:
        _PROGRAM = build_program()
    in_maps = host_inputs(**inputs)
    trace = bool(int(os.environ.get("KERNEL_TRACE", "0")))
    res = run_bass_kernel_spmd(
        _PROGRAM, in_maps, list(range(NCORES)), trace=trace
    )
    LAST_RESULT = res
    out = np.concatenate([res.results[c]["out"] for c in range(NCORES)], axis=0)
    return np.ascontiguousarray(out.astype(np.float32))
